# revision 11
# baseline (speedup 1.0000x reference)
"""Trainium2 Bass kernel: 4096x4096 valid 5x5 cross-correlation + scalar bias.

Strategy (8 NeuronCores, SPMD):
  - Shard the OUTPUT by columns: core c computes out[:, 512c : 512c+512]
    (core 7's last 4 columns are padding, trimmed after gather). Each core
    reads x rows 0..4095, cols [512c, 512c+516) (host-padded to width 4100).
  - On-core: the 5x5 conv is computed as banded-matrix matmuls on the
    TensorEngine. For an input row-tile X_g = x[124g : 124g+128, :] and
    kernel column dj, the banded matrix B_dj[k, m] = w[k-m, dj] gives
      (B_dj^T @ X_g[:, dj:dj+512])[m, n] = sum_di w[di, dj] x[124g+m+di, n+dj]
    so accumulating the 5 dj-matmuls in PSUM yields 124 valid output rows
    per tile. 4092 = 33 * 124 exactly; 33 tiles cover rows 0..4095 exactly.
  - PSUM accumulation is fp32. Bias is fused into the PSUM->SBUF drain
    (ScalarE Identity-activation / VectorE tensor_scalar).
  - Output DMAs are spread across the three descriptor-generation paths
    (sync HWDGE ring: 16 SDMA engines; scalar HWDGE ring: 4 engines;
    gpsimd SWDGE: descgen-limited) with a static schedule so they overlap
    the input stream, which owns the sync ring early in the kernel.
"""
import os

os.environ.setdefault("MYCRO_LOCAL_CACHE", "1")

import numpy as np

import concourse.bass as bass
import concourse.bacc as bacc
import concourse.tile as tile
import concourse.mybir as mybir
from concourse import bass_utils

H, W = 4096, 4096
KH, KW = 5, 5
OH, OW = H - KH + 1, W - KW + 1          # 4092, 4092
NCORES = 8
COLS = 512                               # output cols per core
XC = COLS + KW - 1                       # 516 input cols per core
NG = 33                                  # row tiles per core (33*124 = 4092)
RV = 124                                 # valid output rows per tile
BLK = 3                                  # tiles per PSUM block (3 of 8 banks
                                         # -> two blocks in flight + 1 warmup
                                         # bank, PE never stalls on drains)

_compiled = None
TRACE = False            # test harness can flip this for neuron-profile timing
LAST_EXEC_NS = None

X_DT = "bf16"            # matmul operand dtype: "bf16" | "f32r"
STAGE_BUFS = 6
# Each group's output is written as two DMAs of 64 and 60 rows: the SDMA
# engine fan-out is the largest divisor of the partition count <= 16, so
# 64/60 rows hit 16/15 engines while the naive 124 rows would collapse to 4.
# Chunks are disjoint -> no ordering hazards. Rings rotate per group so no
# single sequencer accumulates the ~0.65us-per-push issue cost.
OUT_RING = ["gpsimd", "scalar", "sync", "gpsimd"]   # indexed by g % 4
OUT_SPLIT = (64, 60)


def _mm_dt():
    return mybir.dt.bfloat16 if X_DT == "bf16" else mybir.dt.float32r


def _build():
    nc = bacc.Bacc("TRN2", target_bir_lowering=False, debug=False,
                   num_devices=NCORES)
    mdt = _mm_dt()

    x_dram = nc.dram_tensor("xs", (H, XC), mdt, kind="ExternalInput")
    b_dram = nc.dram_tensor("bmat", (128, KW * 128), mdt,
                            kind="ExternalInput")
    bias_dram = nc.dram_tensor("biast", (128, 1), mybir.dt.float32,
                               kind="ExternalInput")
    out_dram = nc.dram_tensor("out", (OH, COLS), mybir.dt.bfloat16,
                              kind="ExternalOutput")

    blocks = [list(range(s, min(s + BLK, NG))) for s in range(0, NG, BLK)]
    engs = lambda: {"scalar": nc.scalar, "sync": nc.sync, "gpsimd": nc.gpsimd}

    with tile.TileContext(nc) as tc:
        with (
            tc.tile_pool(name="const", bufs=1) as cpool,
            tc.tile_pool(name="x", bufs=NG) as xpool,
            tc.tile_pool(name="stage", bufs=STAGE_BUFS) as spool,
            tc.tile_pool(name="psum", bufs=6, space=bass.MemorySpace.PSUM) as ppool,
            tc.tile_pool(name="wpsum", bufs=1, space=bass.MemorySpace.PSUM) as wpool,
        ):
            bt = cpool.tile([128, KW * 128], mdt)
            biast = cpool.tile([128, 1], mybir.dt.float32)
            nc.sync.dma_start(bt[:], b_dram.ap())
            nc.scalar.dma_start(biast[:], bias_dram.ap())

            # input row-tiles: tile g holds x rows [124g, 124g+128)
            # Alternate the two HWDGE rings so the 33-tile stream isn't
            # serialized behind one ring's per-DMA issue cost.
            xts = []
            for g in range(NG):
                xt = xpool.tile([128, XC], mdt, tag="x")
                ring = nc.sync if g % 2 == 0 else nc.scalar
                ring.dma_start(xt[:], x_dram.ap()[124 * g:124 * g + 128, :])
                xts.append(xt)

            # HAM warmup: ~4us of junk matmuls on the weight tile while the
            # x stream lands, so the PE clock gate is at 8/8 when real
            # matmuls start. Serialized via WAW on one junk PSUM bank.
            wps = wpool.tile([128, COLS], mybir.dt.float32, name="warm",
                             tag="warm")
            for _ in range(10):
                nc.tensor.matmul(wps[:], bt[:, 0:128],
                                 bt[:, 0:COLS], start=True, stop=True)

            for bi, blk in enumerate(blocks):
                stg = spool.tile([128, len(blk) * COLS], mybir.dt.bfloat16)
                psts = {}
                for g in blk:
                    psts[g] = ppool.tile([128, COLS], mybir.dt.float32,
                                         name=f"ps{g}", tag="ps")
                # weight-stationary sweep: dj outer, tiles inner
                for dj in range(KW):
                    for g in blk:
                        nc.tensor.matmul(
                            psts[g][:],
                            bt[:, dj * 128:(dj + 1) * 128],
                            xts[g][:, dj:dj + COLS],
                            start=(dj == 0),
                            stop=(dj == KW - 1),
                        )
                # drain PSUM -> stage with fused bias, split DVE/ACT
                for i, g in enumerate(blk):
                    dst = stg[0:RV, i * COLS:(i + 1) * COLS]
                    if g % 4 < 3:
                        nc.vector.tensor_scalar_add(dst, psts[g][0:RV, :],
                                                    biast[0:RV, :])
                    else:
                        nc.scalar.activation(dst, psts[g][0:RV, :],
                                             mybir.ActivationFunctionType.Identity,
                                             bias=biast[0:RV, :])
                # output DMAs: per group, two row-chunks with 16/15-engine
                # fan-out, each a contiguous DRAM span
                for i, g in enumerate(blk):
                    ring = engs()[OUT_RING[g % len(OUT_RING)]]
                    r0 = 0
                    for rows in OUT_SPLIT:
                        ring.dma_start(
                            out_dram.ap()[124 * g + r0:124 * g + r0 + rows, :],
                            stg[r0:r0 + rows, i * COLS:(i + 1) * COLS])
                        r0 += rows

    nc.compile()
    return nc


def _banded(weight: np.ndarray) -> np.ndarray:
    ball = np.zeros((128, KW * 128), dtype=np.float32)
    for dj in range(KW):
        for di in range(KH):
            m = np.arange(128 - di)
            ball[m + di, dj * 128 + m] = weight[di, dj]
    return ball


def _to_mm_np(a: np.ndarray) -> np.ndarray:
    if X_DT == "bf16":
        import ml_dtypes
        return a.astype(ml_dtypes.bfloat16)
    return a


def kernel(x: np.ndarray, weight: np.ndarray, bias: np.ndarray) -> np.ndarray:
    global _compiled
    x = np.ascontiguousarray(np.asarray(x, dtype=np.float32))
    weight = np.asarray(weight, dtype=np.float32)
    bias = np.asarray(bias, dtype=np.float32)

    if _compiled is None:
        _compiled = _build()
    nc = _compiled

    xpad = np.zeros((H, NCORES * COLS + KW - 1), dtype=np.float32)
    xpad[:, :W] = x
    xpad = _to_mm_np(xpad)
    ball = _to_mm_np(_banded(weight))
    bias_col = np.full((128, 1), bias[0], dtype=np.float32)

    in_maps = []
    for c in range(NCORES):
        in_maps.append({
            "xs": np.ascontiguousarray(xpad[:, COLS * c: COLS * c + XC]),
            "bmat": ball,
            "biast": bias_col,
        })

    res = bass_utils.run_bass_kernel_spmd(nc, in_maps,
                                          core_ids=list(range(NCORES)),
                                          trace=TRACE)
    global LAST_EXEC_NS
    LAST_EXEC_NS = res.exec_time_ns
    out = np.hstack([np.asarray(res.results[c]["out"], dtype=np.float32)
                     for c in range(NCORES)])
    return np.ascontiguousarray(out[:, :OW])



# revision 16
# speedup vs baseline: 1.1371x; 1.1371x over previous
"""Trainium2 Bass kernel: 4096x4096 valid 5x5 cross-correlation + scalar bias.

Strategy (8 NeuronCores, SPMD):
  - Shard the OUTPUT by columns: core c computes out[:, 512c : 512c+512]
    (core 7's last 4 columns are padding, trimmed after gather). Each core
    reads x rows (padded to 4324), cols [512c, 512c+516) in bf16.
  - On-core: the 5x5 conv runs as PACKED 64x64 tile-position matmuls on the
    TensorEngine. The PE array is addressed as 4 independent 64x64 quadrant
    tiles (tile_position=(64i, 64j)); packed tile-matmuls stream rhs at ~256
    elem/cycle aggregate -- 2x the full-array banded scheme -- and the 64-row
    banded weight blocks waste only 4/64 output rows instead of 4/128.
  - Chunking: chunk = 60 output rows from a 64-row input window. Group = 4
    chunks mapped onto the 4 PE quadrant tiles; supergroup = 2 groups sharing
    one weight load per kernel-column tap dj. PSUM: 2 banks per group -> 4
    supergroups' banks cycle through the 8 banks, so drains overlap matmuls.
  - Output is written SPARSELY to DRAM (each 60-row chunk padded to a 64-row
    strip) so every drain is a full 128-partition copy and every output DMA
    is one contiguous 128-row block; the host strips the 4-row pads for free.
  - PSUM accumulation fp32; bias fused into the PSUM->SBUF drain (split
    across VectorE and ScalarE); drains emit bf16 to halve output DMA bytes.
"""
import os

os.environ.setdefault("MYCRO_LOCAL_CACHE", "1")

import numpy as np

import concourse.bass as bass
import concourse.bacc as bacc
import concourse.tile as tile
import concourse.mybir as mybir
from concourse import bass_utils
from concourse.bass import AP

H, W = 4096, 4096
KH, KW = 5, 5
OH, OW = H - KH + 1, W - KW + 1          # 4092, 4092
NCORES = 8
COLS = 512                               # output cols per core
XC = COLS + KW - 1                       # 516 input cols per core
CH = 60                                  # valid output rows per chunk
CIN = 64                                 # input rows per chunk (CH + KH - 1)
NCH = 72                                 # chunks (69 real, 3 pad)
NGRP = NCH // 4                          # 18 groups of 4 chunks
NSG = NGRP // 2                          # 9 supergroups of 2 groups
XROWS = CH * (NCH - 1) + CIN             # 4324 padded input rows
OROWS = CIN * NCH                        # 4608 sparse output rows

_compiled = None
TRACE = False            # test harness can flip this for neuron-profile timing
LAST_EXEC_NS = None

# ring rotation for the 36 output DMAs (gpsimd SWDGE + both HWDGE rings)
OUT_RING = ["gpsimd", "scalar", "sync", "gpsimd", "sync", "scalar"]


def _build():
    nc = bacc.Bacc("TRN2", target_bir_lowering=False, debug=False,
                   num_devices=NCORES)
    mdt = mybir.dt.bfloat16

    x_dram = nc.dram_tensor("xs", (XROWS, XC), mdt, kind="ExternalInput")
    w_dram = nc.dram_tensor("wmat", (128, KW * 64), mdt,
                            kind="ExternalInput")
    bias_dram = nc.dram_tensor("biast", (128, 1), mybir.dt.float32,
                               kind="ExternalInput")
    out_dram = nc.dram_tensor("out", (OROWS, COLS), mybir.dt.bfloat16,
                              kind="ExternalOutput")

    engs = lambda: {"scalar": nc.scalar, "sync": nc.sync, "gpsimd": nc.gpsimd}

    with tile.TileContext(nc) as tc:
        with (
            tc.tile_pool(name="const", bufs=1) as cpool,
            tc.tile_pool(name="xg", bufs=6) as xpool,
            tc.tile_pool(name="stage", bufs=6) as spool,
            tc.tile_pool(name="psum", bufs=8, space=bass.MemorySpace.PSUM) as ppool,
        ):
            wt = cpool.tile([128, KW * 64], mdt)
            biast = cpool.tile([128, 1], mybir.dt.float32)
            nc.sync.dma_start(wt[:], w_dram.ap())
            nc.scalar.dma_start(biast[:], bias_dram.ap())

            # Input: per group G a [128, 2*516] tile; partition strip i
            # (64 rows) holds chunks 4G+2i and 4G+2i+1 side by side. The
            # DRAM-side AP overlaps rows by 4 (the chunk halo).
            xgs = []
            for G in range(NGRP):
                xt = xpool.tile([128, 2 * XC], mdt, name=f"xg{G}", tag="xg")
                for i in range(2):
                    r0 = CH * (4 * G + 2 * i)
                    src = AP(tensor=x_dram, offset=r0 * XC,
                             ap=[[XC, CIN], [CH * XC, 2], [1, XC]])
                    dst = xt[64 * i:64 * i + 64, :].rearrange(
                        "p (j c) -> p j c", j=2)
                    ring = nc.sync if (2 * G + i) % 2 == 0 else nc.scalar
                    ring.dma_start(dst, src)
                xgs.append(xt)

            od = 0  # output DMA counter (ring rotation)
            for s in range(NSG):
                grps = (2 * s, 2 * s + 1)
                ps = {}
                for gi in range(2):
                    for i in range(2):
                        ps[(gi, i)] = ppool.tile(
                            [128, COLS], mybir.dt.float32,
                            name=f"ps{s}_{gi}_{i}", tag="ps")
                # weight-stationary: dj outer, both groups inside share the
                # 4 quadrant weight loads for this tap
                for dj in range(KW):
                    for gi, G in enumerate(grps):
                        for i in range(2):
                            for j in range(2):
                                nc.tensor.matmul(
                                    ps[(gi, i)][64 * j:64 * j + 64, :],
                                    wt[64 * i:64 * i + 64,
                                       64 * dj:64 * dj + 64],
                                    xgs[G][64 * i:64 * i + 64,
                                           XC * j + dj:XC * j + dj + COLS],
                                    start=(dj == 0),
                                    stop=(dj == KW - 1 and j == 1),
                                    tile_position=(64 * i, 64 * j),
                                )
                # drain (bias fused, bf16 out) + one 128-row output DMA per
                # bank; DVE/ACT alternate so the two engines split the load
                for gi, G in enumerate(grps):
                    for i in range(2):
                        stg = spool.tile([128, COLS], mybir.dt.bfloat16,
                                         name=f"st{s}_{gi}_{i}", tag="st")
                        if (2 * gi + i) % 2 == 0:
                            nc.vector.tensor_scalar_add(stg[:],
                                                        ps[(gi, i)][:],
                                                        biast[:])
                        else:
                            nc.scalar.activation(
                                stg[:], ps[(gi, i)][:],
                                mybir.ActivationFunctionType.Identity,
                                bias=biast[:])
                        r0 = CIN * (4 * G + 2 * i)
                        ring = engs()[OUT_RING[od % len(OUT_RING)]]
                        od += 1
                        ring.dma_start(out_dram.ap()[r0:r0 + 128, :], stg[:])

    nc.compile()
    return nc


def _banded(weight: np.ndarray) -> np.ndarray:
    """[128, 5*64]: strip i (64 rows) holds the five 64x64 banded blocks
    B_dj[k, m] = w[k-m, dj], identical in both strips."""
    ball = np.zeros((128, KW * 64), dtype=np.float32)
    for i in range(2):
        for dj in range(KW):
            for di in range(KH):
                m = np.arange(64 - di)
                ball[64 * i + m + di, 64 * dj + m] = weight[di, dj]
    return ball


def kernel(x: np.ndarray, weight: np.ndarray, bias: np.ndarray) -> np.ndarray:
    global _compiled
    import ml_dtypes
    x = np.asarray(x, dtype=np.float32)
    weight = np.asarray(weight, dtype=np.float32)
    bias = np.asarray(bias, dtype=np.float32)

    if _compiled is None:
        _compiled = _build()
    nc = _compiled

    xpad = np.zeros((XROWS, NCORES * COLS + KW - 1), dtype=np.float32)
    xpad[:H, :W] = x
    xpad = xpad.astype(ml_dtypes.bfloat16)
    ball = _banded(weight).astype(ml_dtypes.bfloat16)
    bias_col = np.full((128, 1), bias[0], dtype=np.float32)

    in_maps = []
    for c in range(NCORES):
        in_maps.append({
            "xs": np.ascontiguousarray(xpad[:, COLS * c: COLS * c + XC]),
            "wmat": ball,
            "biast": bias_col,
        })

    res = bass_utils.run_bass_kernel_spmd(nc, in_maps,
                                          core_ids=list(range(NCORES)),
                                          trace=TRACE)
    global LAST_EXEC_NS
    LAST_EXEC_NS = res.exec_time_ns
    cores = []
    for c in range(NCORES):
        o = np.asarray(res.results[c]["out"], dtype=np.float32)
        o = o.reshape(NCH, CIN, COLS)[:, :CH, :].reshape(NCH * CH, COLS)
        cores.append(o[:OH])
    out = np.hstack(cores)
    return np.ascontiguousarray(out[:, :OW])


# revision 19
# speedup vs baseline: 1.2399x; 1.0903x over previous
"""Trainium2 Bass kernel: 4096x4096 valid 5x5 cross-correlation + scalar bias.

Strategy (8 NeuronCores, SPMD):
  - Shard the OUTPUT by columns: core c computes out[:, 512c : 512c+512]
    (core 7's last 4 columns are padding, trimmed after gather). Each core
    reads x rows (padded to 4324), cols [512c, 512c+516) in bf16.
  - On-core: the 5x5 conv runs as PACKED 64x64 tile-position matmuls on the
    TensorEngine. The PE array is addressed as 4 independent 64x64 quadrant
    tiles (tile_position=(64i, 64j)); packed tile-matmuls stream rhs at ~256
    elem/cycle aggregate -- 2x the full-array banded scheme -- and the 64-row
    banded weight blocks waste only 4/64 output rows instead of 4/128.
  - Chunking: chunk = 60 output rows from a 64-row input window. Group = 4
    chunks mapped onto the 4 PE quadrant tiles; supergroup = 2 groups sharing
    one weight load per kernel-column tap dj. PSUM: 2 banks per group -> 4
    supergroups' banks cycle through the 8 banks, so drains overlap matmuls.
  - Output is written SPARSELY to DRAM (each 60-row chunk padded to a 64-row
    strip) so every drain is a full 128-partition copy and every output DMA
    is one contiguous 128-row block; the host strips the 4-row pads for free.
  - PSUM accumulation fp32; bias fused into the PSUM->SBUF drain (split
    across VectorE and ScalarE); drains emit bf16 to halve output DMA bytes.
"""
import os

os.environ.setdefault("MYCRO_LOCAL_CACHE", "1")

import numpy as np

import concourse.bass as bass
import concourse.bacc as bacc
import concourse.tile as tile
import concourse.mybir as mybir
from concourse import bass_utils
from concourse.bass import AP

H, W = 4096, 4096
KH, KW = 5, 5
OH, OW = H - KH + 1, W - KW + 1          # 4092, 4092
NCORES = 8
COLS = 512                               # output cols per core
XC = COLS + KW - 1                       # 516 input cols per core
CH = 60                                  # valid output rows per chunk
CIN = 64                                 # input rows per chunk (CH + KH - 1)
NCH = 72                                 # chunks (69 real, 3 pad)
NGRP = NCH // 4                          # 18 groups of 4 chunks
NSG = NGRP // 2                          # 9 supergroups of 2 groups
XROWS = CH * (NCH - 1) + CIN             # 4324 padded input rows
OROWS = CIN * NCH                        # 4608 sparse output rows

_compiled = None
TRACE = False            # test harness can flip this for neuron-profile timing
LAST_EXEC_NS = None

# ring rotation for the 36 output DMAs (gpsimd SWDGE + both HWDGE rings)
OUT_RING = ["scalar", "sync", "gpsimd"]


def _build():
    nc = bacc.Bacc("TRN2", target_bir_lowering=False, debug=False,
                   num_devices=NCORES)
    mdt = mybir.dt.bfloat16

    x_dram = nc.dram_tensor("xs", (XROWS, XC), mdt, kind="ExternalInput")
    w_dram = nc.dram_tensor("wmat", (128, KW * 64), mdt,
                            kind="ExternalInput")
    bias_dram = nc.dram_tensor("biast", (128, 1), mybir.dt.float32,
                               kind="ExternalInput")
    out_dram = nc.dram_tensor("out", (OROWS, COLS), mybir.dt.bfloat16,
                              kind="ExternalOutput")

    engs = lambda: {"scalar": nc.scalar, "sync": nc.sync, "gpsimd": nc.gpsimd}

    with tile.TileContext(nc) as tc:
        with (
            tc.tile_pool(name="const", bufs=1) as cpool,
            tc.tile_pool(name="xg", bufs=6) as xpool,
            tc.tile_pool(name="stage", bufs=12) as spool,
            tc.tile_pool(name="psum", bufs=8, space=bass.MemorySpace.PSUM) as ppool,
        ):
            wt = cpool.tile([128, KW * 64], mdt)
            biast = cpool.tile([128, 1], mybir.dt.float32)
            junk = cpool.tile([128, COLS], mdt)
            nc.sync.dma_start(wt[:], w_dram.ap())
            nc.scalar.dma_start(biast[:], bias_dram.ap())

            # HAM warmup: memset a junk tile on-chip (no DMA wait), then run
            # full-array matmuls on it so the PE clock gate is already 8/8
            # when the first real matmul issues (~10us in). The junk PSUM
            # tile joins the "ps" rotation; real MMs overwrite via start=1.
            nc.gpsimd.memset(junk[:], 0)
            wps = ppool.tile([128, COLS], mybir.dt.float32, name="warm",
                             tag="ps")
            for _ in range(8):
                nc.tensor.matmul(wps[:], junk[:, 0:128], junk[:],
                                 start=True, stop=True)

            # Input: per group G a [128, 2*516] tile; partition strip i
            # (64 rows) holds chunks 4G+2i and 4G+2i+1 side by side. The
            # DRAM-side AP overlaps rows by 4 (the chunk halo). Emitted
            # just-in-time (2 supergroups ahead) so the HWDGE ring FIFOs
            # interleave input issues with output issues instead of an
            # output's drain-wait head-of-line-blocking the input stream.
            xgs = [None] * NGRP

            def emit_inputs(s):
                for G in (2 * s, 2 * s + 1):
                    if G >= NGRP:
                        return
                    xt = xpool.tile([128, 2 * XC], mdt, name=f"xg{G}",
                                    tag="xg")
                    for i in range(2):
                        r0 = CH * (4 * G + 2 * i)
                        src = AP(tensor=x_dram, offset=r0 * XC,
                                 ap=[[XC, CIN], [CH * XC, 2], [1, XC]])
                        dst = xt[64 * i:64 * i + 64, :].rearrange(
                            "p (j c) -> p j c", j=2)
                        ring = nc.sync if (2 * G + i) % 2 == 0 else nc.scalar
                        ring.dma_start(dst, src)
                    xgs[G] = xt

            emit_inputs(0)
            emit_inputs(1)

            od = 0  # output DMA counter (ring rotation)
            for s in range(NSG):
                if s + 2 <= NSG - 1:
                    emit_inputs(s + 2)
                grps = (2 * s, 2 * s + 1)
                ps = {}
                for gi in range(2):
                    for i in range(2):
                        ps[(gi, i)] = ppool.tile(
                            [128, COLS], mybir.dt.float32,
                            name=f"ps{s}_{gi}_{i}", tag="ps")
                # weight-stationary: dj outer, both groups inside share the
                # 4 quadrant weight loads for this tap
                for dj in range(KW):
                    for gi, G in enumerate(grps):
                        for i in range(2):
                            for j in range(2):
                                nc.tensor.matmul(
                                    ps[(gi, i)][64 * j:64 * j + 64, :],
                                    wt[64 * i:64 * i + 64,
                                       64 * dj:64 * dj + 64],
                                    xgs[G][64 * i:64 * i + 64,
                                           XC * j + dj:XC * j + dj + COLS],
                                    start=(dj == 0),
                                    stop=(dj == KW - 1 and j == 1),
                                    tile_position=(64 * i, 64 * j),
                                )
                # drain (bias fused, bf16 out) + one 128-row output DMA per
                # bank; DVE/ACT alternate so the two engines split the load
                for gi, G in enumerate(grps):
                    for i in range(2):
                        stg = spool.tile([128, COLS], mybir.dt.bfloat16,
                                         name=f"st{s}_{gi}_{i}", tag="st")
                        if (2 * gi + i) % 2 == 0:
                            nc.vector.tensor_scalar_add(stg[:],
                                                        ps[(gi, i)][:],
                                                        biast[:])
                        else:
                            nc.scalar.activation(
                                stg[:], ps[(gi, i)][:],
                                mybir.ActivationFunctionType.Identity,
                                bias=biast[:])
                        r0 = CIN * (4 * G + 2 * i)
                        ring = engs()[OUT_RING[od % len(OUT_RING)]]
                        od += 1
                        ring.dma_start(out_dram.ap()[r0:r0 + 128, :], stg[:])

    nc.compile()
    return nc


def _banded(weight: np.ndarray) -> np.ndarray:
    """[128, 5*64]: strip i (64 rows) holds the five 64x64 banded blocks
    B_dj[k, m] = w[k-m, dj], identical in both strips."""
    ball = np.zeros((128, KW * 64), dtype=np.float32)
    for i in range(2):
        for dj in range(KW):
            for di in range(KH):
                m = np.arange(64 - di)
                ball[64 * i + m + di, 64 * dj + m] = weight[di, dj]
    return ball


def kernel(x: np.ndarray, weight: np.ndarray, bias: np.ndarray) -> np.ndarray:
    global _compiled
    import ml_dtypes
    x = np.asarray(x, dtype=np.float32)
    weight = np.asarray(weight, dtype=np.float32)
    bias = np.asarray(bias, dtype=np.float32)

    if _compiled is None:
        _compiled = _build()
    nc = _compiled

    xpad = np.zeros((XROWS, NCORES * COLS + KW - 1), dtype=np.float32)
    xpad[:H, :W] = x
    xpad = xpad.astype(ml_dtypes.bfloat16)
    ball = _banded(weight).astype(ml_dtypes.bfloat16)
    bias_col = np.full((128, 1), bias[0], dtype=np.float32)

    in_maps = []
    for c in range(NCORES):
        in_maps.append({
            "xs": np.ascontiguousarray(xpad[:, COLS * c: COLS * c + XC]),
            "wmat": ball,
            "biast": bias_col,
        })

    res = bass_utils.run_bass_kernel_spmd(nc, in_maps,
                                          core_ids=list(range(NCORES)),
                                          trace=TRACE)
    global LAST_EXEC_NS
    LAST_EXEC_NS = res.exec_time_ns
    cores = []
    for c in range(NCORES):
        o = np.asarray(res.results[c]["out"], dtype=np.float32)
        o = o.reshape(NCH, CIN, COLS)[:, :CH, :].reshape(NCH * CH, COLS)
        cores.append(o[:OH])
    out = np.hstack(cores)
    return np.ascontiguousarray(out[:, :OW])


# revision 21
# speedup vs baseline: 1.2885x; 1.0392x over previous
"""Trainium2 Bass kernel: 4096x4096 valid 5x5 cross-correlation + scalar bias.

Strategy (8 NeuronCores, SPMD):
  - Shard the OUTPUT by columns: core c computes out[:, 512c : 512c+512]
    (core 7's last 4 columns are padding, trimmed after gather). Each core
    reads x rows (padded to 4324), cols [512c, 512c+516) in bf16.
  - On-core: the 5x5 conv runs as PACKED 64x64 tile-position matmuls on the
    TensorEngine. The PE array is addressed as 4 independent 64x64 quadrant
    tiles (tile_position=(64i, 64j)); packed tile-matmuls stream rhs at ~256
    elem/cycle aggregate -- 2x the full-array banded scheme -- and the 64-row
    banded weight blocks waste only 4/64 output rows instead of 4/128.
  - Chunking: chunk = 60 output rows from a 64-row input window. Group = 4
    chunks mapped onto the 4 PE quadrant tiles; supergroup = 2 groups sharing
    one weight load per kernel-column tap dj. PSUM: 2 banks per group -> 4
    supergroups' banks cycle through the 8 banks, so drains overlap matmuls.
  - Output is written SPARSELY to DRAM (each 60-row chunk padded to a 64-row
    strip) so every drain is a full 128-partition copy and every output DMA
    is one contiguous 128-row block; the host strips the 4-row pads for free.
  - PSUM accumulation fp32; bias fused into the PSUM->SBUF drain (split
    across VectorE and ScalarE); drains emit bf16 to halve output DMA bytes.
"""
import os

os.environ.setdefault("MYCRO_LOCAL_CACHE", "1")

import numpy as np

import concourse.bass as bass
import concourse.bacc as bacc
import concourse.tile as tile
import concourse.mybir as mybir
from concourse import bass_utils
from concourse.bass import AP

H, W = 4096, 4096
KH, KW = 5, 5
OH, OW = H - KH + 1, W - KW + 1          # 4092, 4092
NCORES = 8
COLS = 512                               # output cols per core
XC = COLS + KW - 1                       # 516 input cols per core
CH = 60                                  # valid output rows per chunk
CIN = 64                                 # input rows per chunk (CH + KH - 1)
NCH = 72                                 # chunks (69 real, 3 pad)
NGRP = NCH // 4                          # 18 groups of 4 chunks
NSG = NGRP // 2                          # 9 supergroups of 2 groups
XROWS = CH * (NCH - 1) + CIN             # 4324 padded input rows
OROWS = CIN * NCH                        # 4608 sparse output rows

_compiled = None
TRACE = False            # test harness can flip this for neuron-profile timing
LAST_EXEC_NS = None

# ring rotation for the 36 output DMAs (gpsimd SWDGE + both HWDGE rings)
OUT_RING = ["scalar", "sync", "gpsimd"]


def _build():
    nc = bacc.Bacc("TRN2", target_bir_lowering=False, debug=False,
                   num_devices=NCORES)
    mdt = mybir.dt.bfloat16

    x_dram = nc.dram_tensor("xs", (XROWS, XC), mdt, kind="ExternalInput")
    w_dram = nc.dram_tensor("wmat", (128, KW * 64), mdt,
                            kind="ExternalInput")
    bias_dram = nc.dram_tensor("biast", (128, 1), mybir.dt.float32,
                               kind="ExternalInput")
    out_dram = nc.dram_tensor("out", (OROWS, COLS), mybir.dt.bfloat16,
                              kind="ExternalOutput")

    engs = lambda: {"scalar": nc.scalar, "sync": nc.sync, "gpsimd": nc.gpsimd}

    with tile.TileContext(nc) as tc:
        with (
            tc.tile_pool(name="const", bufs=1) as cpool,
            tc.tile_pool(name="xg", bufs=8) as xpool,
            tc.tile_pool(name="stage", bufs=12) as spool,
            tc.tile_pool(name="psum", bufs=8, space=bass.MemorySpace.PSUM) as ppool,
        ):
            wt = cpool.tile([128, KW * 64], mdt)
            biast = cpool.tile([128, 1], mybir.dt.float32)
            junk = cpool.tile([128, COLS], mdt)
            nc.sync.dma_start(wt[:], w_dram.ap())
            nc.scalar.dma_start(biast[:], bias_dram.ap())

            # HAM warmup: memset a junk tile on-chip (no DMA wait), then run
            # full-array matmuls on it so the PE clock gate is already 8/8
            # when the first real matmul issues (~10us in). The junk PSUM
            # tile joins the "ps" rotation; real MMs overwrite via start=1.
            nc.gpsimd.memset(junk[:], 0)
            wps = ppool.tile([128, COLS], mybir.dt.float32, name="warm",
                             tag="ps")
            for _ in range(8):
                nc.tensor.matmul(wps[:], junk[:, 0:128], junk[:],
                                 start=True, stop=True)

            # Input: per group G a [128, 2*516] tile; partition strip i
            # (64 rows) holds chunks 4G+2i and 4G+2i+1 side by side. The
            # DRAM-side AP overlaps rows by 4 (the chunk halo). Emitted
            # just-in-time (2 supergroups ahead) so the HWDGE ring FIFOs
            # interleave input issues with output issues instead of an
            # output's drain-wait head-of-line-blocking the input stream.
            xgs = [None] * NGRP

            def emit_inputs(s):
                for G in (2 * s, 2 * s + 1):
                    if G >= NGRP:
                        return
                    xt = xpool.tile([128, 2 * XC], mdt, name=f"xg{G}",
                                    tag="xg")
                    for i in range(2):
                        r0 = CH * (4 * G + 2 * i)
                        src = AP(tensor=x_dram, offset=r0 * XC,
                                 ap=[[XC, CIN], [CH * XC, 2], [1, XC]])
                        dst = xt[64 * i:64 * i + 64, :].rearrange(
                            "p (j c) -> p j c", j=2)
                        ring = nc.sync if (2 * G + i) % 2 == 0 else nc.scalar
                        ring.dma_start(dst, src)
                    xgs[G] = xt

            emit_inputs(0)
            emit_inputs(1)
            emit_inputs(2)

            psall = {}
            state = {"od": 0}

            def emit_drains(s):
                # drains+outputs for supergroup s, emitted one iteration
                # late so output DMAs reach the ring FIFO with their drain
                # semaphores already fired (no head-of-line blocking of the
                # input stream behind a drain-wait)
                for gi, G in enumerate((2 * s, 2 * s + 1)):
                    for i in range(2):
                        stg = spool.tile([128, COLS], mybir.dt.bfloat16,
                                         name=f"st{s}_{gi}_{i}", tag="st")
                        if (2 * gi + i) % 2 == 0:
                            nc.vector.tensor_scalar_add(stg[:],
                                                        psall[(s, gi, i)][:],
                                                        biast[:])
                        else:
                            nc.scalar.activation(
                                stg[:], psall[(s, gi, i)][:],
                                mybir.ActivationFunctionType.Identity,
                                bias=biast[:])
                        r0 = CIN * (4 * G + 2 * i)
                        ring = engs()[OUT_RING[state["od"] % len(OUT_RING)]]
                        state["od"] += 1
                        ring.dma_start(out_dram.ap()[r0:r0 + 128, :], stg[:])

            for s in range(NSG):
                if s + 3 <= NSG - 1:
                    emit_inputs(s + 3)
                grps = (2 * s, 2 * s + 1)
                for gi in range(2):
                    for i in range(2):
                        psall[(s, gi, i)] = ppool.tile(
                            [128, COLS], mybir.dt.float32,
                            name=f"ps{s}_{gi}_{i}", tag="ps")
                # weight-stationary: dj outer, both groups inside share the
                # 4 quadrant weight loads for this tap
                for dj in range(KW):
                    for gi, G in enumerate(grps):
                        for i in range(2):
                            for j in range(2):
                                nc.tensor.matmul(
                                    psall[(s, gi, i)][64 * j:64 * j + 64, :],
                                    wt[64 * i:64 * i + 64,
                                       64 * dj:64 * dj + 64],
                                    xgs[G][64 * i:64 * i + 64,
                                           XC * j + dj:XC * j + dj + COLS],
                                    start=(dj == 0),
                                    stop=(dj == KW - 1 and j == 1),
                                    tile_position=(64 * i, 64 * j),
                                )
                if s >= 1:
                    emit_drains(s - 1)
            emit_drains(NSG - 1)

    nc.compile()
    return nc


def _banded(weight: np.ndarray) -> np.ndarray:
    """[128, 5*64]: strip i (64 rows) holds the five 64x64 banded blocks
    B_dj[k, m] = w[k-m, dj], identical in both strips."""
    ball = np.zeros((128, KW * 64), dtype=np.float32)
    for i in range(2):
        for dj in range(KW):
            for di in range(KH):
                m = np.arange(64 - di)
                ball[64 * i + m + di, 64 * dj + m] = weight[di, dj]
    return ball


def kernel(x: np.ndarray, weight: np.ndarray, bias: np.ndarray) -> np.ndarray:
    global _compiled
    import ml_dtypes
    x = np.asarray(x, dtype=np.float32)
    weight = np.asarray(weight, dtype=np.float32)
    bias = np.asarray(bias, dtype=np.float32)

    if _compiled is None:
        _compiled = _build()
    nc = _compiled

    xpad = np.zeros((XROWS, NCORES * COLS + KW - 1), dtype=np.float32)
    xpad[:H, :W] = x
    xpad = xpad.astype(ml_dtypes.bfloat16)
    ball = _banded(weight).astype(ml_dtypes.bfloat16)
    bias_col = np.full((128, 1), bias[0], dtype=np.float32)

    in_maps = []
    for c in range(NCORES):
        in_maps.append({
            "xs": np.ascontiguousarray(xpad[:, COLS * c: COLS * c + XC]),
            "wmat": ball,
            "biast": bias_col,
        })

    res = bass_utils.run_bass_kernel_spmd(nc, in_maps,
                                          core_ids=list(range(NCORES)),
                                          trace=TRACE)
    global LAST_EXEC_NS
    LAST_EXEC_NS = res.exec_time_ns
    cores = []
    for c in range(NCORES):
        o = np.asarray(res.results[c]["out"], dtype=np.float32)
        o = o.reshape(NCH, CIN, COLS)[:, :CH, :].reshape(NCH * CH, COLS)
        cores.append(o[:OH])
    out = np.hstack(cores)
    return np.ascontiguousarray(out[:, :OW])


# revision 24
# speedup vs baseline: 1.3974x; 1.0846x over previous
"""Trainium2 Bass kernel: 4096x4096 valid 5x5 cross-correlation + scalar bias.

Strategy (8 NeuronCores, SPMD):
  - Shard the OUTPUT by columns: core c computes out[:, 512c : 512c+512]
    (core 7's last 4 columns are padding, trimmed after gather). Each core
    reads x rows (padded to 4324), cols [512c, 512c+516) in bf16.
  - On-core: the 5x5 conv runs as PACKED 64x64 tile-position matmuls on the
    TensorEngine. The PE array is addressed as 4 independent 64x64 quadrant
    tiles (tile_position=(64i, 64j)); packed tile-matmuls stream rhs at ~256
    elem/cycle aggregate -- 2x the full-array banded scheme -- and the 64-row
    banded weight blocks waste only 4/64 output rows instead of 4/128.
  - Chunking: chunk = 60 output rows from a 64-row input window. Group = 4
    chunks mapped onto the 4 PE quadrant tiles; supergroup = 2 groups sharing
    one weight load per kernel-column tap dj. PSUM: 2 banks per group -> 4
    supergroups' banks cycle through the 8 banks, so drains overlap matmuls.
  - Output is written SPARSELY to DRAM (each 60-row chunk padded to a 64-row
    strip) so every drain is a full 128-partition copy and every output DMA
    is one contiguous 128-row block; the host strips the 4-row pads for free.
  - PSUM accumulation fp32; bias fused into the PSUM->SBUF drain (split
    across VectorE and ScalarE); drains emit bf16 to halve output DMA bytes.
"""
import os

os.environ.setdefault("MYCRO_LOCAL_CACHE", "1")

import numpy as np

import concourse.bass as bass
import concourse.bacc as bacc
import concourse.tile as tile
import concourse.mybir as mybir
from concourse import bass_utils
from concourse.bass import AP

H, W = 4096, 4096
KH, KW = 5, 5
OH, OW = H - KH + 1, W - KW + 1          # 4092, 4092
NCORES = 8
COLS = 512                               # output cols per core
XC = COLS + KW - 1                       # 516 input cols per core
CH = 60                                  # valid output rows per chunk
CIN = 64                                 # input rows per chunk (CH + KH - 1)
NCH = 72                                 # chunks (69 real, 3 pad)
NGRP = NCH // 4                          # 18 groups of 4 chunks
NSG = NGRP // 2                          # 9 supergroups of 2 groups
XROWS = CH * (NCH - 1) + CIN             # 4324 padded input rows
OROWS = CIN * NCH                        # 4608 sparse output rows

_compiled = None
TRACE = False            # test harness can flip this for neuron-profile timing
LAST_EXEC_NS = None

# ring rotation for the 36 output DMAs (gpsimd SWDGE + both HWDGE rings)
OUT_RING = ["scalar", "sync", "gpsimd"]


def _build():
    nc = bacc.Bacc("TRN2", target_bir_lowering=False, debug=False,
                   num_devices=NCORES)
    mdt = mybir.dt.bfloat16

    x_dram = nc.dram_tensor("xs", (XROWS, XC), mdt, kind="ExternalInput")
    w_dram = nc.dram_tensor("wmat", (128, KW * 64), mdt,
                            kind="ExternalInput")
    bias_dram = nc.dram_tensor("biast", (128, 1), mybir.dt.float32,
                               kind="ExternalInput")
    out_dram = nc.dram_tensor("out", (OROWS, COLS), mybir.dt.bfloat16,
                              kind="ExternalOutput")

    engs = lambda: {"scalar": nc.scalar, "sync": nc.sync, "gpsimd": nc.gpsimd}

    with tile.TileContext(nc) as tc:
        with (
            tc.tile_pool(name="const", bufs=1) as cpool,
            tc.tile_pool(name="xg", bufs=5) as xpool,
            tc.tile_pool(name="stage", bufs=12) as spool,
            tc.tile_pool(name="psum", bufs=8, space=bass.MemorySpace.PSUM) as ppool,
        ):
            wt = cpool.tile([128, KW * 64], mdt)
            biast = cpool.tile([128, 1], mybir.dt.float32)
            junk = cpool.tile([128, COLS], mdt)
            nc.sync.dma_start(wt[:], w_dram.ap())
            nc.scalar.dma_start(biast[:], bias_dram.ap())

            # HAM warmup: memset a junk tile on-chip (no DMA wait), then run
            # full-array matmuls on it so the PE clock gate is already 8/8
            # when the first real matmul issues (~10us in). The junk PSUM
            # tile joins the "ps" rotation; real MMs overwrite via start=1.
            nc.gpsimd.memset(junk[:], 0)
            wps = ppool.tile([128, COLS], mybir.dt.float32, name="warm",
                             tag="ps")
            for _ in range(8):
                nc.tensor.matmul(wps[:], junk[:, 0:128], junk[:],
                                 start=True, stop=True)

            # Input: one [128, 4*516] tile per SUPERGROUP; partition strip i
            # (64 rows) holds chunks 8s+4i .. 8s+4i+3 side by side (the
            # DRAM-side AP overlaps rows by 4, the chunk halo). Two 264KB
            # DMAs per supergroup: HWDGE ring issue cost is flat ~600ns per
            # dma_start, so bigger transfers keep the input stream ahead of
            # the PE. Emitted just-in-time so ring FIFOs stay interleaved.
            xgs = [None] * NSG

            def emit_inputs(s):
                if s >= NSG:
                    return
                xt = xpool.tile([128, 4 * XC], mdt, name=f"xg{s}", tag="xg")
                for i in range(2):
                    r0 = CH * (8 * s + 4 * i)
                    src = AP(tensor=x_dram, offset=r0 * XC,
                             ap=[[XC, CIN], [CH * XC, 4], [1, XC]])
                    dst = xt[64 * i:64 * i + 64, :].rearrange(
                        "p (q c) -> p q c", q=4)
                    ring = nc.sync if i == 0 else nc.scalar
                    ring.dma_start(dst, src)
                xgs[s] = xt

            emit_inputs(0)
            emit_inputs(1)
            emit_inputs(2)

            psall = {}
            state = {"od": 0}

            def emit_drains(s):
                # drains+outputs for supergroup s, emitted one iteration
                # late so output DMAs reach the ring FIFO with their drain
                # semaphores already fired (no head-of-line blocking of the
                # input stream behind a drain-wait)
                for gi in range(2):
                    for i in range(2):
                        stg = spool.tile([128, COLS], mybir.dt.bfloat16,
                                         name=f"st{s}_{gi}_{i}", tag="st")
                        if (2 * gi + i) % 2 == 0:
                            nc.vector.tensor_scalar_add(stg[:],
                                                        psall[(s, gi, i)][:],
                                                        biast[:])
                        else:
                            nc.scalar.activation(
                                stg[:], psall[(s, gi, i)][:],
                                mybir.ActivationFunctionType.Identity,
                                bias=biast[:])
                        r0 = CIN * (8 * s + 4 * i + 2 * gi)
                        ring = engs()[OUT_RING[state["od"] % len(OUT_RING)]]
                        state["od"] += 1
                        ring.dma_start(out_dram.ap()[r0:r0 + 128, :], stg[:])

            for s in range(NSG):
                if s + 3 <= NSG - 1:
                    emit_inputs(s + 3)
                for gi in range(2):
                    for i in range(2):
                        psall[(s, gi, i)] = ppool.tile(
                            [128, COLS], mybir.dt.float32,
                            name=f"ps{s}_{gi}_{i}", tag="ps")
                # weight-stationary: dj outer, both bank-pairs inside share
                # the 4 quadrant weight loads for this tap. Bank (gi, i)
                # col-group j holds chunk 8s + 4i + 2gi + j.
                for dj in range(KW):
                    for gi in range(2):
                        for i in range(2):
                            for j in range(2):
                                q = 2 * gi + j
                                nc.tensor.matmul(
                                    psall[(s, gi, i)][64 * j:64 * j + 64, :],
                                    wt[64 * i:64 * i + 64,
                                       64 * dj:64 * dj + 64],
                                    xgs[s][64 * i:64 * i + 64,
                                           XC * q + dj:XC * q + dj + COLS],
                                    start=(dj == 0),
                                    stop=(dj == KW - 1 and j == 1),
                                    tile_position=(64 * i, 64 * j),
                                )
                if s >= 1:
                    emit_drains(s - 1)
            emit_drains(NSG - 1)

    nc.compile()
    return nc


def _banded(weight: np.ndarray) -> np.ndarray:
    """[128, 5*64]: strip i (64 rows) holds the five 64x64 banded blocks
    B_dj[k, m] = w[k-m, dj], identical in both strips."""
    ball = np.zeros((128, KW * 64), dtype=np.float32)
    for i in range(2):
        for dj in range(KW):
            for di in range(KH):
                m = np.arange(64 - di)
                ball[64 * i + m + di, 64 * dj + m] = weight[di, dj]
    return ball


def kernel(x: np.ndarray, weight: np.ndarray, bias: np.ndarray) -> np.ndarray:
    global _compiled
    import ml_dtypes
    x = np.asarray(x, dtype=np.float32)
    weight = np.asarray(weight, dtype=np.float32)
    bias = np.asarray(bias, dtype=np.float32)

    if _compiled is None:
        _compiled = _build()
    nc = _compiled

    xpad = np.zeros((XROWS, NCORES * COLS + KW - 1), dtype=np.float32)
    xpad[:H, :W] = x
    xpad = xpad.astype(ml_dtypes.bfloat16)
    ball = _banded(weight).astype(ml_dtypes.bfloat16)
    bias_col = np.full((128, 1), bias[0], dtype=np.float32)

    in_maps = []
    for c in range(NCORES):
        in_maps.append({
            "xs": np.ascontiguousarray(xpad[:, COLS * c: COLS * c + XC]),
            "wmat": ball,
            "biast": bias_col,
        })

    res = bass_utils.run_bass_kernel_spmd(nc, in_maps,
                                          core_ids=list(range(NCORES)),
                                          trace=TRACE)
    global LAST_EXEC_NS
    LAST_EXEC_NS = res.exec_time_ns
    cores = []
    for c in range(NCORES):
        o = np.asarray(res.results[c]["out"], dtype=np.float32)
        o = o.reshape(NCH, CIN, COLS)[:, :CH, :].reshape(NCH * CH, COLS)
        cores.append(o[:OH])
    out = np.hstack(cores)
    return np.ascontiguousarray(out[:, :OW])
